# revision 12
# baseline (speedup 1.0000x reference)
"""Causal self-attention (B=4, T=2048, C=1024, H=16) on 8 TRN2 NeuronCores.

Sharding: tensor-parallel over heads. Core c owns heads (2c, 2c+1) for all
batches: QKV projections are column-sharded, attention is embarrassingly
parallel over (batch, head), out_proj is contraction-sharded and the host
sums the 8 partial outputs (the unshard step for a contraction shard).

Per-core kernel (all GEMMs bf16 operands, fp32 PSUM accumulation), built as
a software pipeline over batches so the PE / ACT / DVE engines overlap:

  - Per batch b: phase B (QKV projections + RoPE + V transpose), phase C
    (attention), phase D (out_proj partials).  Emission interleaves phase
    B(b+1) and D(b-1) groups into phase C(b)'s attention steps so the PE
    stream never drains (keeps the PE p-state at full clock) while ACT
    (softmax exp) runs in parallel.
  - x^T [C, B*T] bf16 in DRAM, one fused DMA per 512-token tile.
  - Q^T/K^T feature-major with host-permuted RoPE layout (even dims in
    partitions [0:32), odd in [32:64) per head).  RoPE:
      t_a = (q + bq) * [cos|sin|...],  t_b = (q + bq) * [sin|cos|...]
    on DVE, rotation combine rot = Ca^T t_a + Cb^T t_b on PE.
  - V feature-major then PE-transposed to token-major blocks with an
    appended ones column (PV emits O^T rows + softmax denominator at once).
    bv is folded into the host-side bias (bo' = bo + bv @ Wo).
  - Attention: S^T[k,q] tiles per (head, q-tile of 512); k-blocks of 128
    processed in pairs sharing one [128,1024] 2-bank PSUM tile and ONE
    fused exp on ACT.  The causal staircase is exploited at 128-col
    granularity: diagonal k-blocks compute/exp/PV only the valid q-columns,
    and the [128,128] triangle mask multiply runs on the (otherwise idle)
    GpSimd engine.  S runs one pair-step ahead of PV so PE never waits for
    ACT.
  - out_proj partials: wo-block stationary; PSUM results copied (DVE/ACT
    alternating) into paired [128,2,512] bf16 tiles, one DMA per ob-pair.
  - outT is bf16 (partials summed in fp32 on the host).

Host: sums the 8 partials, adds bo + bv@Wo, transposes back to (B, T, C).
"""

import numpy as np
import ml_dtypes

import concourse.bass as bass
import concourse.mybir as mybir
import concourse.tile as tile
from concourse import bacc
from concourse.bass_utils import run_bass_kernel_spmd
from concourse.masks import make_identity

BF16 = mybir.dt.bfloat16
F32 = mybir.dt.float32
AT = mybir.ActivationFunctionType
OP = mybir.AluOpType

B, T, C, H = 4, 2048, 1024, 16
DH = 64
BT = B * T            # 8192
NCORES = 8
NKB = T // 128        # 16 k-blocks per batch
NTT = 4               # token tiles of 512 per batch

_NC = None            # cached compiled Bass module


def _build_nc(repeat=1, phases="BCD"):
    nc = bacc.Bacc("TRN2", target_bir_lowering=False, debug=False)

    xT = nc.declare_dram_parameter("xT", [C, BT], BF16, isOutput=False)
    wq = nc.declare_dram_parameter("wq", [C, 128], BF16, isOutput=False)
    wk = nc.declare_dram_parameter("wk", [C, 128], BF16, isOutput=False)
    wv = nc.declare_dram_parameter("wv", [C, 128], BF16, isOutput=False)
    wo = nc.declare_dram_parameter("wo", [128, C], BF16, isOutput=False)
    bq = nc.declare_dram_parameter("bq", [128, 1], F32, isOutput=False)
    bk = nc.declare_dram_parameter("bk", [128, 1], F32, isOutput=False)
    csa = nc.declare_dram_parameter("csa", [128, T], F32, isOutput=False)
    csb = nc.declare_dram_parameter("csb", [128, T], F32, isOutput=False)
    msk = nc.declare_dram_parameter("msk", [128, 128], BF16, isOutput=False)
    ca = nc.declare_dram_parameter("ca", [128, 128], BF16, isOutput=False)
    cb = nc.declare_dram_parameter("cb", [128, 128], BF16, isOutput=False)
    outT = nc.declare_dram_parameter("outT", [C, BT], BF16, isOutput=True)

    xTr = xT.rearrange("(kb p) m -> p kb m", p=128)
    outTr = outT.rearrange("(g p) m -> p g m", p=128)

    from contextlib import ExitStack
    with tile.TileContext(nc) as tc, ExitStack() as ctx:
        const = ctx.enter_context(tc.tile_pool(name="const", bufs=1))
        xpool = ctx.enter_context(tc.tile_pool(name="xpool", bufs=4))
        qkv = ctx.enter_context(tc.tile_pool(name="qkv", bufs=2))
        rtmp = ctx.enter_context(tc.tile_pool(name="rtmp", bufs=4))
        ptp = ctx.enter_context(tc.tile_pool(name="ptp", bufs=3))
        sm = ctx.enter_context(tc.tile_pool(name="sm", bufs=2))
        dst = ctx.enter_context(tc.tile_pool(name="dst", bufs=3))
        psA = ctx.enter_context(tc.tile_pool(name="psA", bufs=2, space="PSUM"))
        psS = ctx.enter_context(tc.tile_pool(name="psS", bufs=2, space="PSUM"))
        psO = ctx.enter_context(tc.tile_pool(name="psO", bufs=2, space="PSUM"))

        # ---- constants ----
        wq_sb = const.tile([128, 8, 128], BF16, tag="wq")
        nc.sync.dma_start(out=wq_sb, in_=wq.rearrange("(kb p) m -> p kb m", p=128))
        wk_sb = const.tile([128, 8, 128], BF16, tag="wk")
        nc.sync.dma_start(out=wk_sb, in_=wk.rearrange("(kb p) m -> p kb m", p=128))
        wv_sb = const.tile([128, 8, 128], BF16, tag="wv")
        nc.sync.dma_start(out=wv_sb, in_=wv.rearrange("(kb p) m -> p kb m", p=128))
        wo_sb = const.tile([128, 8, 128], BF16, tag="wo")
        nc.sync.dma_start(out=wo_sb, in_=wo.rearrange("p (ob m) -> p ob m", m=128))
        csa_sb = const.tile([128, T], F32, tag="csa")
        nc.sync.dma_start(out=csa_sb, in_=csa[:, :])
        csb_sb = const.tile([128, T], F32, tag="csb")
        nc.sync.dma_start(out=csb_sb, in_=csb[:, :])
        msk_sb = const.tile([128, 128], BF16, tag="msk")
        nc.sync.dma_start(out=msk_sb, in_=msk[:, :])
        ca_sb = const.tile([128, 128], BF16, tag="ca")
        nc.sync.dma_start(out=ca_sb, in_=ca[:, :])
        cb_sb = const.tile([128, 128], BF16, tag="cb")
        nc.sync.dma_start(out=cb_sb, in_=cb[:, :])
        bq_sb = const.tile([128, 1], F32, tag="bq")
        nc.sync.dma_start(out=bq_sb, in_=bq[:, :])
        bk_sb = const.tile([128, 1], F32, tag="bk")
        nc.sync.dma_start(out=bk_sb, in_=bk[:, :])

        ident = const.tile([128, 64], BF16, tag="id")
        make_identity(nc, ident[0:64, :])
        make_identity(nc, ident[64:128, :])

        def emit_stream(nbatches):
            # per-batch double-buffered activations; batch index n is global
            # across repeat bodies (n % B selects the data slice) so bodies
            # pipeline into each other and the tail cost is paid only once.
            QTb, KTb, VTb, yTb, vtmb, xts = {}, {}, {}, {}, {}, {}

            def alloc_bt(b):
                QTb[b] = qkv.tile([128, T], BF16, tag="QT", name=f"QT_{b}")
                KTb[b] = qkv.tile([128, T], BF16, tag="KT", name=f"KT_{b}")
                VTb[b] = qkv.tile([128, T], BF16, tag="VT", name=f"VT_{b}")
                yTb[b] = qkv.tile([128, T], BF16, tag="yT", name=f"yT_{b}")
                vtmb[b] = qkv.tile([128, 2, NKB, 66], BF16, tag="vtm",
                                   name=f"vtm_{b}")
                nc.vector.memset(vtmb[b][:, :, :, 64:65], 1.0)

            def g_load(b, tt):
                def go():
                    xt = xpool.tile([128, 8, 512], BF16, tag="xt",
                                    name=f"xt_{b}_{tt}")
                    t0 = ((b % B) * NTT + tt) * 512
                    nc.sync.dma_start(out=xt, in_=xTr[:, :, t0:t0 + 512])
                    xts[(b, tt)] = xt
                return go

            # deferred RoPE state: proj -> (ta, tb, b_sb, dstT)
            rope_q = {}

            def proj_chain(b, tt, w_sb):
                pp = psA.tile([128, 512], F32, tag="proj", name=f"pp_{b}_{tt}")
                xt = xts[(b, tt)]
                for kb in range(8):
                    nc.tensor.matmul(pp, w_sb[:, kb, :], xt[:, kb, :],
                                     start=(kb == 0), stop=(kb == 7))
                return pp

            def rope_stt(b, tt, pp, b_sb, key):
                pos = slice(tt * 512, tt * 512 + 512)
                ta = rtmp.tile([128, 512], BF16, tag="t", name=f"ta_{b}_{tt}")
                tb = rtmp.tile([128, 512], BF16, tag="t", name=f"tb_{b}_{tt}")
                nc.vector.scalar_tensor_tensor(
                    out=ta, in0=pp, scalar=b_sb[:, 0:1], in1=csa_sb[:, pos],
                    op0=OP.add, op1=OP.mult)
                nc.vector.scalar_tensor_tensor(
                    out=tb, in0=pp, scalar=b_sb[:, 0:1], in1=csb_sb[:, pos],
                    op0=OP.add, op1=OP.mult)
                rope_q[key] = (ta, tb)

            def rope_combine(b, tt, key, dstT):
                ta, tb = rope_q.pop(key)
                rp = psA.tile([128, 512], F32, tag="proj", name=f"rp_{b}_{tt}")
                nc.tensor.matmul(rp, ca_sb, ta, start=True, stop=False)
                nc.tensor.matmul(rp, cb_sb, tb, start=False, stop=True)
                nc.vector.tensor_copy(dstT[:, tt * 512:tt * 512 + 512], rp)

            def g_projQ(b, tt):
                def go():
                    pp = proj_chain(b, tt, wq_sb)
                    rope_stt(b, tt, pp, bq_sb, ("q", b, tt))
                return go

            def g_projK(b, tt):
                def go():
                    pp = proj_chain(b, tt, wk_sb)
                    rope_stt(b, tt, pp, bk_sb, ("k", b, tt))
                    rope_combine(b, tt, ("q", b, tt), QTb[b])
                return go

            def g_projV(b, tt):
                def go():
                    pp = proj_chain(b, tt, wv_sb)
                    rope_combine(b, tt, ("k", b, tt), KTb[b])
                    nc.vector.tensor_copy(
                        VTb[b][:, tt * 512:tt * 512 + 512], pp)
                return go

            def g_vtrans(b, tt):
                def go():
                    for j in range(2):
                        tp = psA.tile([128, 256], BF16, tag="proj",
                                      name=f"tp_{b}_{tt}_{j}")
                        for sub in range(4):
                            col = slice(tt * 512 + sub * 128,
                                        tt * 512 + sub * 128 + 128)
                            nc.tensor.transpose(
                                tp[:, sub * 64:sub * 64 + 64],
                                VTb[b][64 * j:64 * j + 64, col],
                                ident[64 * j:64 * j + 64, :])
                        nc.vector.tensor_copy(
                            vtmb[b][:, j, tt * 4:tt * 4 + 4, 0:64],
                            tp.rearrange("p (s d) -> p s d", d=64))
                return go

            def g_dstore(b, tt, m):
                # out_proj for token-tile tt, output blocks (2m, 2m+1)
                def go():
                    ot = dst.tile([128, 2, 512], BF16, tag="ot",
                                  name=f"ot_{b}_{tt}_{m}")
                    for i in range(2):
                        ob = 2 * m + i
                        pp = psO.tile([128, 512], F32, tag="o",
                                      name=f"op_{b}_{tt}_{ob}")
                        nc.tensor.matmul(pp, wo_sb[:, ob, :],
                                         yTb[b][:, tt * 512:tt * 512 + 512],
                                         start=True, stop=True)
                        if (tt + i) % 2 == 0:
                            nc.vector.tensor_copy(ot[:, i, :], pp)
                        else:
                            nc.scalar.copy(ot[:, i, :], pp)
                    t0 = ((b % B) * NTT + tt) * 512
                    nc.sync.dma_start(
                        out=outTr[:, 2 * m:2 * m + 2, t0:t0 + 512], in_=ot)
                return go

            def b_groups(b):
                gs = [g_load(b, tt) for tt in range(NTT)]
                for tt in range(NTT):
                    gs += [g_projQ(b, tt), g_projK(b, tt),
                           g_projV(b, tt), g_vtrans(b, tt)]
                return gs

            def d_groups(b):
                return [g_dstore(b, tt, m)
                        for tt in range(NTT) for m in range(4)]

            def c_batch(b, fillers, late_d=None):
                def pop_filler():
                    if fillers:
                        fillers.pop(0)()

                for j in range(2):
                    hsl = slice(64 * j, 64 * j + 64)
                    for qt in range(4):
                        nkb = 4 * qt + 4
                        prs = [(2 * i, 2 * i + 1) for i in range(2 * qt + 2)]
                        op = psO.tile([128, 512], F32, tag="o",
                                      name=f"o_{b}_{j}_{qt}")

                        def emit_pv(state):
                            pt, pr, Ws = state
                            for half, kb in enumerate(pr):
                                W = Ws[half]
                                o = (512 - W) if half == 0 else 512
                                nc.tensor.matmul(
                                    op[0:65, 512 - W:512],
                                    vtmb[b][:, j, kb, 0:65],
                                    pt[:, o:o + W],
                                    start=(kb == 0), stop=(kb == nkb - 1))

                        prev = None
                        for pr in prs:
                            sp = psS.tile([128, 1024], F32, tag="s",
                                          name=f"s_{b}_{j}_{qt}")
                            pt = ptp.tile([128, 1024], BF16, tag="pt",
                                          name=f"pt_{b}_{j}_{qt}")
                            Ws = []
                            for half, kb in enumerate(pr):
                                # half 0 sits at its bank's end, half 1 at
                                # its bank's start, so the fused exp range
                                # [512-W0 : 512+W1] is contiguous-valid
                                W = (512 if kb < 4 * qt
                                     else 512 - (kb - 4 * qt) * 128)
                                o = (512 - W) if half == 0 else 512
                                nc.tensor.matmul(
                                    sp[:, o:o + W],
                                    KTb[b][hsl, kb * 128:kb * 128 + 128],
                                    QTb[b][hsl, qt * 512 + 512 - W:
                                           qt * 512 + 512],
                                    start=True, stop=True)
                                Ws.append(W)
                            lo = 512 - Ws[0]
                            nc.scalar.activation(
                                pt[:, lo:512 + Ws[1]], sp[:, lo:512 + Ws[1]],
                                AT.Exp, scale=0.125)
                            for half, kb in enumerate(pr):
                                if kb >= 4 * qt:
                                    base = (512 - Ws[0]) if half == 0 else 512
                                    nc.gpsimd.tensor_tensor(
                                        out=pt[:, base:base + 128],
                                        in0=pt[:, base:base + 128],
                                        in1=msk_sb, op=OP.mult)
                            if prev is not None:
                                emit_pv(prev)
                            prev = (pt, pr, Ws)
                            pop_filler()
                        emit_pv(prev)

                        recip = sm.tile([1, 512], F32, tag="rc",
                                        name=f"rc_{b}_{j}_{qt}")
                        nc.vector.reciprocal(recip, op[64:65, :])
                        rbt = sm.tile([64, 512], F32, tag="rb",
                                      name=f"rb_{b}_{j}_{qt}")
                        nc.gpsimd.partition_broadcast(rbt, recip)
                        nc.vector.tensor_tensor(
                            out=yTb[b][hsl, qt * 512:qt * 512 + 512],
                            in0=op[0:64, :], in1=rbt, op=OP.mult)
                        pop_filler()
                        if late_d is not None and j == 1:
                            # D(b) groups for this q-tile become available
                            fillers += late_d(qt)
                while fillers:
                    fillers.pop(0)()

            # ---- pipeline over the global batch stream ----
            alloc_bt(0)
            for g in b_groups(0):
                g()
            dlast = d_groups(nbatches - 1)
            for b in range(nbatches):
                fillers = []
                if b + 1 < nbatches:
                    alloc_bt(b + 1)
                    bg = b_groups(b + 1)
                    fillers += bg[:NTT]       # xt loads first
                    bg = bg[NTT:]
                else:
                    bg = []
                dg = d_groups(b - 1) if b >= 1 else []
                # interleave B and D groups
                n = max(len(bg), len(dg))
                for i in range(n):
                    if i < len(bg):
                        fillers.append(bg[i])
                    if i < len(dg):
                        fillers.append(dg[i])
                late = None
                if b == nbatches - 1:
                    late = lambda qt: dlast[qt * 4:qt * 4 + 4]
                c_batch(b, fillers, late_d=late)

        emit_stream(B * repeat)

    nc.compile()
    return nc


def _get_nc():
    global _NC
    if _NC is None:
        _NC = _build_nc()
    return _NC


def _prep_in_maps(x, Wq, bq, Wk, bk, Wv, bv, Wo, bo):
    bf = ml_dtypes.bfloat16
    # x^T, bf16-rounded (matches reference's x.astype(bf16) exactly)
    xT = np.ascontiguousarray(
        np.asarray(x, np.float32).reshape(BT, C).astype(bf).T
    )

    # RoPE caches; rows [cos|sin|cos|sin] and [sin|cos|sin|cos]
    inv = (1.0 / 10000.0 ** (np.arange(0, DH, 2, dtype=np.float64) / DH))
    pos = np.arange(T, dtype=np.float64)
    fr = np.outer(pos, inv)                      # [T, 32]
    cosT = np.cos(fr).T.astype(np.float32)       # [32, T]
    sinT = np.sin(fr).T.astype(np.float32)
    csa = np.ascontiguousarray(np.concatenate([cosT, sinT, cosT, sinT], 0))
    csb = np.ascontiguousarray(np.concatenate([sinT, cosT, sinT, cosT], 0))

    # causal triangle mask for the first 128 columns of each diagonal k-block
    ki = np.arange(128)[:, None]
    qi = np.arange(128)[None, :]
    msk = np.ascontiguousarray((qi >= ki).astype(bf))      # [128, 128]

    # RoPE combine matrices: rot = Ca^T t_a + Cb^T t_b
    ca = np.zeros((128, 128), np.float32)
    cb = np.zeros((128, 128), np.float32)
    for base in (0, 64):
        for m in range(32):
            ca[base + m, base + m] = 1.0          # E*cos
            ca[base + m + 32, base + m] = -1.0    # -O*sin
            cb[base + m, base + m + 32] = 1.0     # E*sin
            cb[base + m + 32, base + m + 32] = 1.0  # O*cos
    ca = ca.astype(bf)
    cb = cb.astype(bf)

    perm = np.concatenate([np.arange(0, DH, 2), np.arange(1, DH, 2)])
    Wq = np.asarray(Wq, np.float32)
    Wk = np.asarray(Wk, np.float32)
    Wv = np.asarray(Wv, np.float32)
    Wo = np.asarray(Wo, np.float32)
    bq = np.asarray(bq, np.float32)
    bk = np.asarray(bk, np.float32)

    in_maps = []
    for c in range(NCORES):
        h0, h1 = 2 * c, 2 * c + 1
        cols = np.concatenate([DH * h0 + perm, DH * h1 + perm])
        in_maps.append({
            "xT": xT,
            "wq": np.ascontiguousarray(Wq[:, cols].astype(bf)),
            "wk": np.ascontiguousarray(Wk[:, cols].astype(bf)),
            "wv": np.ascontiguousarray(Wv[:, 128 * c:128 * c + 128].astype(bf)),
            "wo": np.ascontiguousarray(Wo[128 * c:128 * c + 128, :].astype(bf)),
            "bq": np.ascontiguousarray(bq[cols].reshape(128, 1)),
            "bk": np.ascontiguousarray(bk[cols].reshape(128, 1)),
            "csa": csa, "csb": csb, "msk": msk, "ca": ca, "cb": cb,
        })
    return in_maps


def _gather(results, bo, bv, Wo):
    acc = results[0]["outT"].astype(np.float32)
    for c in range(1, NCORES):
        acc = acc + results[c]["outT"].astype(np.float32)
    # bv never enters the device: y = PV/d + bv, so out += bv @ Wo (+ bo)
    bias = (np.asarray(bo, np.float32)
            + np.asarray(bv, np.float32) @ np.asarray(Wo, np.float32))
    out = acc.T.reshape(B, T, C) + bias
    return np.ascontiguousarray(out.astype(np.float32))


def kernel(x, Wq, bq, Wk, bk, Wv, bv, Wo, bo):
    nc = _get_nc()
    in_maps = _prep_in_maps(x, Wq, bq, Wk, bk, Wv, bv, Wo, bo)
    res = run_bass_kernel_spmd(nc, in_maps, list(range(NCORES)))
    return _gather(res.results, bo, bv, Wo)


# revision 18
# speedup vs baseline: 2.6241x; 2.6241x over previous
"""Causal self-attention (B=4, T=2048, C=1024, H=16) on 8 TRN2 NeuronCores.

Sharding: tensor-parallel over heads. Core c owns heads (2c, 2c+1) for all
batches: QKV projections are column-sharded, attention is embarrassingly
parallel over (batch, head), out_proj is contraction-sharded and the host
sums the 8 partial outputs (the unshard step for a contraction shard).

Per-core kernel (all GEMMs bf16 operands, fp32 PSUM accumulation), built as
a software pipeline over batches so the PE / ACT / DVE engines overlap:

  - Per batch b: phase B (QKV projections + RoPE + V transpose), phase C
    (attention), phase D (out_proj partials).  Emission interleaves phase
    B(b+1) and D(b-1) groups into phase C(b)'s attention steps so the PE
    stream never drains (keeps the PE p-state at full clock) while ACT
    (softmax exp) runs in parallel.
  - x^T [C, B*T] bf16 in DRAM, one fused DMA per 512-token tile.
  - Q^T/K^T feature-major with host-permuted RoPE layout (even dims in
    partitions [0:32), odd in [32:64) per head).  RoPE:
      t_a = (q + bq) * [cos|sin|...],  t_b = (q + bq) * [sin|cos|...]
    on DVE, rotation combine rot = Ca^T t_a + Cb^T t_b on PE.
  - V feature-major then PE-transposed to token-major blocks with an
    appended ones column (PV emits O^T rows + softmax denominator at once).
    bv is folded into the host-side bias (bo' = bo + bv @ Wo).
  - Attention: S^T[k,q] tiles per (head, q-tile of 512); k-blocks of 128
    processed in pairs sharing one [128,1024] 2-bank PSUM tile and ONE
    fused exp on ACT.  The causal staircase is exploited at 128-col
    granularity: diagonal k-blocks compute/exp/PV only the valid q-columns,
    and the [128,128] triangle mask multiply runs on the (otherwise idle)
    GpSimd engine.  S runs one pair-step ahead of PV so PE never waits for
    ACT.
  - out_proj partials: wo-block stationary; PSUM results copied (DVE/ACT
    alternating) into paired [128,2,512] bf16 tiles, one DMA per ob-pair.
  - outT is bf16 (partials summed in fp32 on the host).

Host: sums the 8 partials, adds bo + bv@Wo, transposes back to (B, T, C).
"""

import numpy as np
import ml_dtypes

import concourse.bass as bass
import concourse.mybir as mybir
import concourse.tile as tile
from concourse import bacc
from concourse.bass_utils import run_bass_kernel_spmd
from concourse.masks import make_identity

BF16 = mybir.dt.bfloat16
F32 = mybir.dt.float32
AT = mybir.ActivationFunctionType
OP = mybir.AluOpType

B, T, C, H = 4, 2048, 1024, 16
DH = 64
BT = B * T            # 8192
NCORES = 8
NKB = T // 128        # 16 k-blocks per batch
NTT = 4               # token tiles of 512 per batch

_NC = None            # cached compiled Bass module


def _build_nc(repeat=1, phases="BCD"):
    nc = bacc.Bacc("TRN2", target_bir_lowering=False, debug=False)

    xT = nc.declare_dram_parameter("xT", [C, BT], BF16, isOutput=False)
    wq = nc.declare_dram_parameter("wq", [C, 128], BF16, isOutput=False)
    wk = nc.declare_dram_parameter("wk", [C, 128], BF16, isOutput=False)
    wv = nc.declare_dram_parameter("wv", [C, 128], BF16, isOutput=False)
    wo = nc.declare_dram_parameter("wo", [128, C], BF16, isOutput=False)
    bq = nc.declare_dram_parameter("bq", [128, 1], F32, isOutput=False)
    bk = nc.declare_dram_parameter("bk", [128, 1], F32, isOutput=False)
    csa = nc.declare_dram_parameter("csa", [128, T], F32, isOutput=False)
    csb = nc.declare_dram_parameter("csb", [128, T], F32, isOutput=False)
    msk = nc.declare_dram_parameter("msk", [128, 128], BF16, isOutput=False)
    ca = nc.declare_dram_parameter("ca", [128, 128], BF16, isOutput=False)
    cb = nc.declare_dram_parameter("cb", [128, 128], BF16, isOutput=False)
    outT = nc.declare_dram_parameter("outT", [C, BT], BF16, isOutput=True)

    xTr = xT.rearrange("(kb p) m -> p kb m", p=128)
    outTr = outT.rearrange("(g p) m -> p g m", p=128)

    from contextlib import ExitStack
    with tile.TileContext(nc) as tc, ExitStack() as ctx:
        const = ctx.enter_context(tc.tile_pool(name="const", bufs=1))
        xpool = ctx.enter_context(tc.tile_pool(name="xpool", bufs=4))
        qkv = ctx.enter_context(tc.tile_pool(name="qkv", bufs=2))
        rtmp = ctx.enter_context(tc.tile_pool(name="rtmp", bufs=4))
        ptp = ctx.enter_context(tc.tile_pool(name="ptp", bufs=3))
        sm = ctx.enter_context(tc.tile_pool(name="sm", bufs=2))
        dst = ctx.enter_context(tc.tile_pool(name="dst", bufs=3))
        psA = ctx.enter_context(tc.tile_pool(name="psA", bufs=2, space="PSUM"))
        psS = ctx.enter_context(tc.tile_pool(name="psS", bufs=2, space="PSUM"))
        psO = ctx.enter_context(tc.tile_pool(name="psO", bufs=2, space="PSUM"))

        # ---- constants ----
        wq_sb = const.tile([128, 8, 128], BF16, tag="wq")
        nc.sync.dma_start(out=wq_sb, in_=wq.rearrange("(kb p) m -> p kb m", p=128))
        wk_sb = const.tile([128, 8, 128], BF16, tag="wk")
        nc.sync.dma_start(out=wk_sb, in_=wk.rearrange("(kb p) m -> p kb m", p=128))
        wv_sb = const.tile([128, 8, 128], BF16, tag="wv")
        nc.sync.dma_start(out=wv_sb, in_=wv.rearrange("(kb p) m -> p kb m", p=128))
        wo_sb = const.tile([128, 8, 128], BF16, tag="wo")
        nc.sync.dma_start(out=wo_sb, in_=wo.rearrange("p (ob m) -> p ob m", m=128))
        csa_sb = const.tile([128, T], F32, tag="csa")
        nc.sync.dma_start(out=csa_sb, in_=csa[:, :])
        csb_sb = const.tile([128, T], F32, tag="csb")
        nc.sync.dma_start(out=csb_sb, in_=csb[:, :])
        msk_sb = const.tile([128, 128], BF16, tag="msk")
        nc.sync.dma_start(out=msk_sb, in_=msk[:, :])
        ca_sb = const.tile([128, 128], BF16, tag="ca")
        nc.sync.dma_start(out=ca_sb, in_=ca[:, :])
        cb_sb = const.tile([128, 128], BF16, tag="cb")
        nc.sync.dma_start(out=cb_sb, in_=cb[:, :])
        bq_sb = const.tile([128, 1], F32, tag="bq")
        nc.sync.dma_start(out=bq_sb, in_=bq[:, :])
        bk_sb = const.tile([128, 1], F32, tag="bk")
        nc.sync.dma_start(out=bk_sb, in_=bk[:, :])

        ident = const.tile([128, 64], BF16, tag="id")
        make_identity(nc, ident[0:64, :])
        make_identity(nc, ident[64:128, :])

        def emit_stream(nbatches):
            # per-batch double-buffered activations; batch index n is global
            # across repeat bodies (n % B selects the data slice) so bodies
            # pipeline into each other and the tail cost is paid only once.
            QTb, KTb, VTb, yTb, vtmb, xts = {}, {}, {}, {}, {}, {}

            def alloc_bt(b):
                QTb[b] = qkv.tile([128, T], BF16, tag="QT", name=f"QT_{b}")
                KTb[b] = qkv.tile([128, T], BF16, tag="KT", name=f"KT_{b}")
                VTb[b] = qkv.tile([128, T], BF16, tag="VT", name=f"VT_{b}")
                yTb[b] = qkv.tile([128, T], BF16, tag="yT", name=f"yT_{b}")
                vtmb[b] = qkv.tile([128, 2, NKB, 66], BF16, tag="vtm",
                                   name=f"vtm_{b}")
                nc.vector.memset(vtmb[b][:, :, :, 64:65], 1.0)

            def g_load(b, tt):
                def go():
                    xt = xpool.tile([128, 8, 512], BF16, tag="xt",
                                    name=f"xt_{b}_{tt}")
                    t0 = ((b % B) * NTT + tt) * 512
                    nc.sync.dma_start(out=xt, in_=xTr[:, :, t0:t0 + 512])
                    xts[(b, tt)] = xt
                return go

            # deferred RoPE state: proj -> (ta, tb, b_sb, dstT)
            rope_q = {}

            def proj_chain(b, tt, w_sb):
                pp = psA.tile([128, 512], F32, tag="proj", name=f"pp_{b}_{tt}")
                xt = xts[(b, tt)]
                for kb in range(8):
                    nc.tensor.matmul(pp, w_sb[:, kb, :], xt[:, kb, :],
                                     start=(kb == 0), stop=(kb == 7))
                return pp

            def rope_stt(b, tt, pp, b_sb, key):
                pos = slice(tt * 512, tt * 512 + 512)
                ta = rtmp.tile([128, 512], BF16, tag="t", name=f"ta_{b}_{tt}")
                tb = rtmp.tile([128, 512], BF16, tag="t", name=f"tb_{b}_{tt}")
                nc.vector.scalar_tensor_tensor(
                    out=ta, in0=pp, scalar=b_sb[:, 0:1], in1=csa_sb[:, pos],
                    op0=OP.add, op1=OP.mult)
                nc.vector.scalar_tensor_tensor(
                    out=tb, in0=pp, scalar=b_sb[:, 0:1], in1=csb_sb[:, pos],
                    op0=OP.add, op1=OP.mult)
                rope_q[key] = (ta, tb)

            def rope_combine(b, tt, key, dstT):
                ta, tb = rope_q.pop(key)
                rp = psA.tile([128, 512], F32, tag="proj", name=f"rp_{b}_{tt}")
                nc.tensor.matmul(rp, ca_sb, ta, start=True, stop=False)
                nc.tensor.matmul(rp, cb_sb, tb, start=False, stop=True)
                nc.vector.tensor_copy(dstT[:, tt * 512:tt * 512 + 512], rp)

            def g_projQ(b, tt):
                def go():
                    pp = proj_chain(b, tt, wq_sb)
                    rope_stt(b, tt, pp, bq_sb, ("q", b, tt))
                return go

            def g_projK(b, tt):
                def go():
                    pp = proj_chain(b, tt, wk_sb)
                    rope_stt(b, tt, pp, bk_sb, ("k", b, tt))
                    rope_combine(b, tt, ("q", b, tt), QTb[b])
                return go

            def g_projV(b, tt):
                def go():
                    pp = proj_chain(b, tt, wv_sb)
                    rope_combine(b, tt, ("k", b, tt), KTb[b])
                    nc.vector.tensor_copy(
                        VTb[b][:, tt * 512:tt * 512 + 512], pp)
                return go

            def g_vtrans(b, tt):
                def go():
                    for j in range(2):
                        tp = psA.tile([128, 256], BF16, tag="proj",
                                      name=f"tp_{b}_{tt}_{j}")
                        for sub in range(4):
                            col = slice(tt * 512 + sub * 128,
                                        tt * 512 + sub * 128 + 128)
                            nc.tensor.transpose(
                                tp[:, sub * 64:sub * 64 + 64],
                                VTb[b][64 * j:64 * j + 64, col],
                                ident[64 * j:64 * j + 64, :])
                        nc.vector.tensor_copy(
                            vtmb[b][:, j, tt * 4:tt * 4 + 4, 0:64],
                            tp.rearrange("p (s d) -> p s d", d=64))
                return go

            def g_dstore(b, tt, m):
                # out_proj for token-tile tt, output blocks (2m, 2m+1)
                def go():
                    ot = dst.tile([128, 2, 512], BF16, tag="ot",
                                  name=f"ot_{b}_{tt}_{m}")
                    for i in range(2):
                        ob = 2 * m + i
                        pp = psO.tile([128, 512], F32, tag="o",
                                      name=f"op_{b}_{tt}_{ob}")
                        nc.tensor.matmul(pp, wo_sb[:, ob, :],
                                         yTb[b][:, tt * 512:tt * 512 + 512],
                                         start=True, stop=True)
                        if (tt + i) % 2 == 0:
                            nc.vector.tensor_copy(ot[:, i, :], pp)
                        else:
                            nc.scalar.copy(ot[:, i, :], pp)
                    t0 = ((b % B) * NTT + tt) * 512
                    nc.sync.dma_start(
                        out=outTr[:, 2 * m:2 * m + 2, t0:t0 + 512], in_=ot)
                return go

            def b_groups(b):
                gs = [g_load(b, tt) for tt in range(NTT)]
                for tt in range(NTT):
                    gs += [g_projQ(b, tt), g_projK(b, tt),
                           g_projV(b, tt), g_vtrans(b, tt)]
                return gs

            def d_groups(b):
                return [g_dstore(b, tt, m)
                        for tt in range(NTT) for m in range(4)]

            def c_batch(b, fillers, late_d=None):
                def pop_filler():
                    if fillers:
                        fillers.pop(0)()

                for j in range(2):
                    hsl = slice(64 * j, 64 * j + 64)
                    for qt in range(4):
                        nkb = 4 * qt + 4
                        prs = [(2 * i, 2 * i + 1) for i in range(2 * qt + 2)]
                        op = psO.tile([128, 512], F32, tag="o",
                                      name=f"o_{b}_{j}_{qt}")

                        def emit_pv(state):
                            pt, pr, Ws = state
                            for half, kb in enumerate(pr):
                                W = Ws[half]
                                o = (512 - W) if half == 0 else 512
                                nc.tensor.matmul(
                                    op[0:65, 512 - W:512],
                                    vtmb[b][:, j, kb, 0:65],
                                    pt[:, o:o + W],
                                    start=(kb == 0), stop=(kb == nkb - 1))

                        prev = None
                        for pr in prs:
                            sp = psS.tile([128, 1024], F32, tag="s",
                                          name=f"s_{b}_{j}_{qt}")
                            pt = ptp.tile([128, 1024], BF16, tag="pt",
                                          name=f"pt_{b}_{j}_{qt}")
                            Ws = []
                            for half, kb in enumerate(pr):
                                # half 0 sits at its bank's end, half 1 at
                                # its bank's start, so the fused exp range
                                # [512-W0 : 512+W1] is contiguous-valid
                                W = (512 if kb < 4 * qt
                                     else 512 - (kb - 4 * qt) * 128)
                                o = (512 - W) if half == 0 else 512
                                nc.tensor.matmul(
                                    sp[:, o:o + W],
                                    KTb[b][hsl, kb * 128:kb * 128 + 128],
                                    QTb[b][hsl, qt * 512 + 512 - W:
                                           qt * 512 + 512],
                                    start=True, stop=True)
                                Ws.append(W)
                            lo = 512 - Ws[0]
                            nc.scalar.activation(
                                pt[:, lo:512 + Ws[1]], sp[:, lo:512 + Ws[1]],
                                AT.Exp, scale=0.125)
                            for half, kb in enumerate(pr):
                                if kb >= 4 * qt:
                                    base = (512 - Ws[0]) if half == 0 else 512
                                    nc.vector.tensor_tensor(
                                        out=pt[:, base:base + 128],
                                        in0=pt[:, base:base + 128],
                                        in1=msk_sb, op=OP.mult)
                            if prev is not None:
                                emit_pv(prev)
                            prev = (pt, pr, Ws)
                            pop_filler()
                        emit_pv(prev)

                        recip = sm.tile([1, 512], F32, tag="rc",
                                        name=f"rc_{b}_{j}_{qt}")
                        nc.vector.reciprocal(recip, op[64:65, :])
                        rbt = sm.tile([64, 512], F32, tag="rb",
                                      name=f"rb_{b}_{j}_{qt}")
                        nc.gpsimd.partition_broadcast(rbt, recip)
                        nc.vector.tensor_tensor(
                            out=yTb[b][hsl, qt * 512:qt * 512 + 512],
                            in0=op[0:64, :], in1=rbt, op=OP.mult)
                        pop_filler()
                        if late_d is not None and j == 1:
                            # D(b) groups for this q-tile become available
                            fillers += late_d(qt)
                while fillers:
                    fillers.pop(0)()

            # ---- pipeline over the global batch stream ----
            alloc_bt(0)
            for g in b_groups(0):
                g()
            dlast = d_groups(nbatches - 1)
            for b in range(nbatches):
                fillers = []
                if b + 1 < nbatches:
                    alloc_bt(b + 1)
                    bg = b_groups(b + 1)
                    fillers += bg[:NTT]       # xt loads first
                    bg = bg[NTT:]
                else:
                    bg = []
                dg = d_groups(b - 1) if b >= 1 else []
                # interleave B and D groups
                n = max(len(bg), len(dg))
                for i in range(n):
                    if i < len(bg):
                        fillers.append(bg[i])
                    if i < len(dg):
                        fillers.append(dg[i])
                late = None
                if b == nbatches - 1:
                    late = lambda qt: dlast[qt * 4:qt * 4 + 4]
                c_batch(b, fillers, late_d=late)

        emit_stream(B * repeat)

    nc.compile()
    return nc


def _get_nc():
    global _NC
    if _NC is None:
        _NC = _build_nc()
    return _NC


def _prep_in_maps(x, Wq, bq, Wk, bk, Wv, bv, Wo, bo):
    bf = ml_dtypes.bfloat16
    # x^T, bf16-rounded (matches reference's x.astype(bf16) exactly)
    xT = np.ascontiguousarray(
        np.asarray(x, np.float32).reshape(BT, C).astype(bf).T
    )

    # RoPE caches; rows [cos|sin|cos|sin] and [sin|cos|sin|cos]
    inv = (1.0 / 10000.0 ** (np.arange(0, DH, 2, dtype=np.float64) / DH))
    pos = np.arange(T, dtype=np.float64)
    fr = np.outer(pos, inv)                      # [T, 32]
    cosT = np.cos(fr).T.astype(np.float32)       # [32, T]
    sinT = np.sin(fr).T.astype(np.float32)
    csa = np.ascontiguousarray(np.concatenate([cosT, sinT, cosT, sinT], 0))
    csb = np.ascontiguousarray(np.concatenate([sinT, cosT, sinT, cosT], 0))

    # causal triangle mask for the first 128 columns of each diagonal k-block
    ki = np.arange(128)[:, None]
    qi = np.arange(128)[None, :]
    msk = np.ascontiguousarray((qi >= ki).astype(bf))      # [128, 128]

    # RoPE combine matrices: rot = Ca^T t_a + Cb^T t_b
    ca = np.zeros((128, 128), np.float32)
    cb = np.zeros((128, 128), np.float32)
    for base in (0, 64):
        for m in range(32):
            ca[base + m, base + m] = 1.0          # E*cos
            ca[base + m + 32, base + m] = -1.0    # -O*sin
            cb[base + m, base + m + 32] = 1.0     # E*sin
            cb[base + m + 32, base + m + 32] = 1.0  # O*cos
    ca = ca.astype(bf)
    cb = cb.astype(bf)

    perm = np.concatenate([np.arange(0, DH, 2), np.arange(1, DH, 2)])
    Wq = np.asarray(Wq, np.float32)
    Wk = np.asarray(Wk, np.float32)
    Wv = np.asarray(Wv, np.float32)
    Wo = np.asarray(Wo, np.float32)
    bq = np.asarray(bq, np.float32)
    bk = np.asarray(bk, np.float32)

    in_maps = []
    for c in range(NCORES):
        h0, h1 = 2 * c, 2 * c + 1
        cols = np.concatenate([DH * h0 + perm, DH * h1 + perm])
        in_maps.append({
            "xT": xT,
            "wq": np.ascontiguousarray(Wq[:, cols].astype(bf)),
            "wk": np.ascontiguousarray(Wk[:, cols].astype(bf)),
            "wv": np.ascontiguousarray(Wv[:, 128 * c:128 * c + 128].astype(bf)),
            "wo": np.ascontiguousarray(Wo[128 * c:128 * c + 128, :].astype(bf)),
            "bq": np.ascontiguousarray(bq[cols].reshape(128, 1)),
            "bk": np.ascontiguousarray(bk[cols].reshape(128, 1)),
            "csa": csa, "csb": csb, "msk": msk, "ca": ca, "cb": cb,
        })
    return in_maps


def _gather(results, bo, bv, Wo):
    acc = results[0]["outT"].astype(np.float32)
    for c in range(1, NCORES):
        acc = acc + results[c]["outT"].astype(np.float32)
    # bv never enters the device: y = PV/d + bv, so out += bv @ Wo (+ bo)
    bias = (np.asarray(bo, np.float32)
            + np.asarray(bv, np.float32) @ np.asarray(Wo, np.float32))
    out = acc.T.reshape(B, T, C) + bias
    return np.ascontiguousarray(out.astype(np.float32))


def kernel(x, Wq, bq, Wk, bk, Wv, bv, Wo, bo):
    nc = _get_nc()
    in_maps = _prep_in_maps(x, Wq, bq, Wk, bk, Wv, bv, Wo, bo)
    res = run_bass_kernel_spmd(nc, in_maps, list(range(NCORES)))
    return _gather(res.results, bo, bv, Wo)
